# revision 2
# baseline (speedup 1.0000x reference)
"""MinimalRNNCell Trainium2 kernel (8 NeuronCores) — bf16 I/O + 4-step blocks.

Math:  h_t = x_t @ K + h_{t-1} @ R,  h_0 = 0, return all h_t  [B, T, U].

Strategy
--------
1. TIME-shard across the 8 cores (256 output steps each).  R is strongly
   contractive (||R^8||_2 ~ 1e-3 with transients), so each core recomputes
   a W=8 step warmup from h=0; truncation error is far below bf16 noise.
2. 4-step BLOCK recurrence: for block start t0 (h_b = h_{t0-1}):
       h_{t0+i} = sum_{j<=i} (K R^j)^T x_{t0+i-j}  +  (R^{i+1})^T h_b
   The only serial dependency is ONE PSUM->SBUF copy (slot 3) per 4 steps,
   which amortizes the ~1.2us matmul->sem->copy->sem latency loop that
   gates shorter-stride pipelines.  Blocks consume only in-block x, so
   chunks need no overlap columns.  Matmul outputs are batched per weight
   but split at the PSUM bank boundary (walrus rejects cross-bank outputs).
3. All HBM traffic in bf16 (x, y, weights); PSUM accumulation fp32.
   All 8 weights ship as ONE packed [D, 8, U] tensor in one DMA (the
   sliced weight APs also give LDWEIGHTS a clean unit-stride pattern,
   which the tensor engine hides under the previous matmul).
4. Transposed layout: state is [U=128 part, B=256 free]; host feeds x
   pre-transposed per core ([D, TP, B]) and re-transposes outputs; the
   device does zero transposes and every DMA is contiguous.
5. Edge trims: chunk 0 is the 8 warmup steps only, DMA'd per-block so
   compute starts after a 262KB transfer; the last chunk's output goes
   out in per-block (and final per-pair) DMAs so the tail transfer after
   the last copy is small.
"""


import sys

import numpy as np

if "/opt/trn_rl_repo" not in sys.path:
    sys.path.insert(0, "/opt/trn_rl_repo")

B, T, D, U = 256, 2048, 128, 128
NCORES = 8
W = 8               # warmup steps recomputed per core (contractive truncation)
TC = T // NCORES    # 256 output steps per core
TP = TC + W         # 264 processed steps per core
CH = 16             # steps per steady-state chunk (chunk 0 is the W warmup)
L = 4               # steps per recurrence block

_PROGRAM = None     # cached bass program


def _build_program():
    import concourse.bacc as bacc
    import concourse.mybir as mybir
    import concourse.tile as tile

    f32 = mybir.dt.float32
    bf16 = mybir.dt.bfloat16
    nc = bacc.Bacc("TRN2", target_bir_lowering=False)

    xT = nc.dram_tensor("xT", [D, TP, B], bf16, kind="ExternalInput")
    # cols 0..3 = K R^j (x-weights), cols 4..7 = R^(j-3) (boundary weights)
    wd = nc.dram_tensor("wd", [D, 2 * L, U], bf16, kind="ExternalInput")
    yT = nc.dram_tensor("yT", [U, TC, B], bf16, kind="ExternalOutput")

    n_chunks = 1 + TC // CH     # warmup chunk + 16 output chunks
    with tile.TileContext(nc) as tc:
        with (
            tc.tile_pool(name="wpool", bufs=1) as wpool,
            tc.tile_pool(name="xpool", bufs=4) as xpool,
            tc.tile_pool(name="ypool", bufs=3) as ypool,
            tc.tile_pool(name="psum", bufs=3, space="PSUM") as pp,
        ):
            w_sb = wpool.tile([D, 2 * L, U], bf16)
            nc.sync.dma_start(w_sb[:], wd[:])
            ws = [w_sb[:, j, :] for j in range(L)]
            rs = [w_sb[:, L + i, :] for i in range(L)]

            prev_y = None
            for c in range(n_chunks):
                cw = W if c == 0 else CH            # chunk width in steps
                x0 = 0 if c == 0 else W + (c - 1) * CH  # first col in xT
                x_sb = xpool.tile([D, cw, B], bf16)
                if c <= 1:
                    # Per-block DMAs so compute starts after 262KB.
                    for blk in range(cw // L):
                        nc.sync.dma_start(
                            x_sb[:, blk * L : (blk + 1) * L, :],
                            xT[:, x0 + blk * L : x0 + (blk + 1) * L, :],
                        )
                else:
                    nc.sync.dma_start(x_sb[:], xT[:, x0 : x0 + cw, :])
                y_sb = ypool.tile([U, cw, B], bf16)
                for blk in range(cw // L):
                    j0 = blk * L            # block-local step base in chunk
                    ps = pp.tile([U, L, B], f32, tag="ps")
                    if c == 0 and blk == 0:
                        # First block of the core: h_{-1}=0, no boundary
                        # terms; per-slot 256-wide matmuls for clean flags.
                        for i in range(L):
                            for j in range(i + 1):
                                nc.tensor.matmul(
                                    ps[:, i, :], ws[j], x_sb[:, i - j, :],
                                    start=(j == 0), stop=(j == i),
                                )
                    else:
                        # x-projections, batched per weight but split at the
                        # PSUM bank boundary (slots 0,1 = bank 0; 2,3 =
                        # bank 1).
                        nc.tensor.matmul(    # w0 -> slots 0,1
                            ps[:, 0:2, :], ws[0], x_sb[:, j0 : j0 + 2, :],
                            start=True, stop=False,
                        )
                        nc.tensor.matmul(    # w0 -> slots 2,3
                            ps[:, 2:4, :], ws[0], x_sb[:, j0 + 2 : j0 + 4, :],
                            start=True, stop=False,
                        )
                        nc.tensor.matmul(    # w1 -> slot 1
                            ps[:, 1, :], ws[1], x_sb[:, j0, :],
                            start=False, stop=False,
                        )
                        nc.tensor.matmul(    # w1 -> slots 2,3
                            ps[:, 2:4, :], ws[1], x_sb[:, j0 + 1 : j0 + 3, :],
                            start=False, stop=False,
                        )
                        nc.tensor.matmul(    # w2 -> slots 2,3
                            ps[:, 2:4, :], ws[2], x_sb[:, j0 : j0 + 2, :],
                            start=False, stop=False,
                        )
                        nc.tensor.matmul(    # w3 -> slot 3
                            ps[:, 3, :], ws[3], x_sb[:, j0, :],
                            start=False, stop=False,
                        )
                        hb = (
                            y_sb[:, j0 - 1, :]
                            if j0 >= 1
                            else prev_y[:, prev_cw - 1, :]
                        )
                        # Boundary terms; R^4 first so the slot-3 copy (the
                        # only cross-block dependency) fires earliest.
                        nc.tensor.matmul(
                            ps[:, 3, :], rs[3], hb, start=False, stop=True
                        )
                        nc.tensor.matmul(
                            ps[:, 2, :], rs[2], hb, start=False, stop=True
                        )
                        nc.tensor.matmul(
                            ps[:, 1, :], rs[1], hb, start=False, stop=True
                        )
                        nc.tensor.matmul(
                            ps[:, 0, :], rs[0], hb, start=False, stop=True
                        )
                    # slot 3 feeds the next block's boundary matmuls: copy
                    # it first (DVE); spread the rest across DVE/Act.
                    nc.vector.tensor_copy(y_sb[:, j0 + 3, :], ps[:, 3, :])
                    nc.scalar.copy(y_sb[:, j0 + 2, :], ps[:, 2, :])
                    nc.vector.tensor_copy(y_sb[:, j0 + 1, :], ps[:, 1, :])
                    nc.scalar.copy(y_sb[:, j0, :], ps[:, 0, :])
                    if c == n_chunks - 1:
                        # Tail: per-pair DMAs right behind the copies keep
                        # the post-compute transfer small.
                        o0 = (c - 1) * CH + j0
                        nc.sync.dma_start(
                            yT[:, o0 + 2 : o0 + 4, :],
                            y_sb[:, j0 + 2 : j0 + 4, :],
                        )
                        nc.sync.dma_start(
                            yT[:, o0 : o0 + 2, :], y_sb[:, j0 : j0 + 2, :]
                        )
                if 1 <= c < n_chunks - 1:
                    nc.sync.dma_start(
                        yT[:, (c - 1) * CH : c * CH, :], y_sb[:]
                    )
                prev_y = y_sb
                prev_cw = cw

    nc.compile()
    return nc


def _get_program():
    global _PROGRAM
    if _PROGRAM is None:
        _PROGRAM = _build_program()
    return _PROGRAM


def _shard_inputs(x, k, r):
    import ml_dtypes

    bf = np.dtype(ml_dtypes.bfloat16)
    xTfull = np.transpose(x, (2, 1, 0)).astype(bf)  # [D, T, B] bf16
    k64 = np.asarray(k, dtype=np.float64)
    r64 = np.asarray(r, dtype=np.float64)
    wd = np.empty((D, 2 * L, U), np.float32)
    rj = np.eye(U)
    for j in range(L):
        wd[:, j, :] = (k64 @ rj).astype(np.float32)      # K R^j
        rj = rj @ r64
        wd[:, L + j, :] = rj.astype(np.float32)          # R^(j+1)
    wd = wd.astype(bf)
    in_maps = []
    for c in range(NCORES):
        buf = np.empty((D, TP, B), bf)
        s = c * TC - W  # timestep of column 0
        if c == 0:
            buf[:, :W, :] = np.zeros((), bf)
            buf[:, W:, :] = xTfull[:, :TC, :]
        else:
            buf[:, :, :] = xTfull[:, s : s + TP, :]
        in_maps.append({"xT": buf, "wd": wd})
    return in_maps


def run(inputs, trace=False, trace_cores=None):
    """Run the kernel; returns (y_full, BassKernelResults)."""
    from concourse import bass_utils

    x = np.ascontiguousarray(inputs["x"], dtype=np.float32)
    k = inputs["kernel"]
    r = inputs["recurrent_kernel"]
    assert x.shape == (B, T, D), x.shape

    nc = _get_program()
    in_maps = _shard_inputs(x, k, r)

    kwargs = {}
    if trace:
        # Profiling writes NTFFs locally; skip the artifact upload step.
        bass_utils.upload_artifacts = lambda tmpdir: tmpdir
        kwargs["trace"] = True
        if trace_cores is not None:
            kwargs["trace_cores"] = trace_cores

    import time as _time

    for attempt in range(3):
        try:
            res = bass_utils.run_bass_kernel_spmd(
                nc, in_maps, core_ids=list(range(NCORES)), **kwargs
            )
            break
        except Exception:  # noqa: BLE001
            if attempt == 2:
                raise
            _time.sleep(20.0 * (attempt + 1))
            if attempt == 1:
                try:
                    import jax

                    jax.clear_caches()
                    from jax._src import xla_bridge

                    xla_bridge._clear_backends()
                except Exception:  # noqa: BLE001
                    pass

    y = np.empty((B, T, U), np.float32)
    for c, out in enumerate(res.results):
        y[:, c * TC : (c + 1) * TC, :] = np.transpose(
            out["yT"].astype(np.float32), (2, 1, 0)
        )
    return y, res


def kernel(**inputs) -> np.ndarray:
    y, _ = run(inputs, trace=False)
    return y
